# revision 39
# baseline (speedup 1.0000x reference)
"""Fused single-head attention with query-sum output, for 8 Trainium2 cores.

Reference computation (per batch b of 16):
    q = x @ Wq + bq ; k = x @ Wk + bk ; v = x @ Wv + bv        [S, D]
    energy = q @ k.T / sqrt(D)                                  [S, S]
    attn   = softmax(energy, axis=-1)
    out    = (attn @ v).sum(axis=0)                             [D]

Key algebraic restructuring: out = colsum @ v_nobias + S * bv, where
colsum[k] = sum_q attn[q, k] = sum_q w[q] * E[q, k] with E = exp(energy)
and w[q] = 1 / sum_k E[q, k].  This replaces the O(S^2 D) attn @ v matmul
with an O(S^2) weighted column reduction (done on the PE with w as the
stationary operand) plus a single matvec against v.  Max-subtraction in
the softmax is skipped: logits are ~N(0, 1) by construction, far inside
exp's fp32 range.

Sharding: pure data-parallel over the batch dim — 2 batches per core on
8 cores, full (tiny) weights replicated.  No collectives.

Device layout per batch (P = 128 partitions):
    xT  [P, 2, S]  bf16   x transposed (contraction dim on partitions);
                          transpose + cast on host so device DMAs are
                          plain copies.
    qT  [P, 2, S]  fp8e4  4*q, d on partitions (planar: DoubleRow
                          LDWEIGHTS needs non-degenerate pair strides)
    kT  [P, S, 2]  fp8e4  4*k, INTERLEAVED pair layout so the DoubleRow
                          moving operand's dependency bounding-box stays
                          narrow (planar made every energy matmul wait
                          for ALL k evacuations)
    v   [P, 16, D] bf16   v natural (s on partitions)
    per 128-query tile: energy as fp8 DoubleRow matmuls (full D=256
    contraction packed per pass — measured-equal per-instruction cost to
    bf16, so half the energy instructions), exp on ScalarE with fused
    per-row accumulation (Z), w = 1/Z on VectorE, then one PE pass per
    tile accumulates w.T @ E into colsum, whose 4 512-wide slices are
    packed into partition rows 0/32/64/96 of a single PSUM bank.

q/k quantization to fp8 at x4 scale happens AFTER bf16 projections, at
PSUM evacuation (scale+bias fused into the evac op; batch-1's evacs run
on DVE so ScalarE stays pure-exp during attention).  Everything feeding
the output directly (v, E, w, colsum, colT) stays bf16: the output is a
random-sign sum over k, so per-element noise there does not average out
(fp8 v measured 3.7e-2 total error vs 6.0e-3 for this scheme; the gate
is 2e-2).
"""

import numpy as np
import ml_dtypes

import concourse.bass as bass
import concourse.mybir as mybir
import concourse.tile as tile
from concourse.bass import ts, ds
from concourse.bass_utils import run_bass_kernel_spmd

B, S, D = 16, 2048, 256
N_CORES = 8
BPC = B // N_CORES          # batches per core
P = 128
CC = D // P                 # contraction chunks over d (2)
DT = D // P                 # output-d tiles (2)
ST = S // P                 # 128-row tiles of the sequence (16)
NS = S // 512               # 512-wide slices of the sequence (4)
F32 = mybir.dt.float32
BF16 = mybir.dt.bfloat16
FP8 = mybir.dt.float8e4
EXP = mybir.ActivationFunctionType.Exp
IDENT = mybir.ActivationFunctionType.Identity
MULT = mybir.AluOpType.mult
ADD = mybir.AluOpType.add
DR = mybir.MatmulPerfMode.DoubleRow
INV_SQRT_D = 1.0 / np.sqrt(D)
# q/k are quantized to fp8e4 at x4 scale after the bf16 projections
# (validated ~6e-3 total error vs the 2e-2 gate); the energy matmuls
# then run fp8 DoubleRow, packing the full D=256 contraction per pass.
QK_SCALE = 4.0
EXP_SCALE = INV_SQRT_D / (QK_SCALE * QK_SCALE)

_MAX_WAITS = 1  # this container's walrus rejects >1 sync wait per instruction


def _split_wide_waits(nc, max_waits=_MAX_WAITS):
    """walrus CoreV3 codegen here rejects instructions with more than one
    sync wait ("Too many sync wait commands").  Move excess waits onto
    freshly inserted same-engine NoOps placed immediately before the wide
    instruction (engine program order preserves semantics)."""
    n_split = 0
    for f in nc.m.functions:
        for blk in f.blocks:
            out = []
            changed = False
            for ins in blk.instructions:
                si = ins.sync_info
                if si is not None and len(si.on_wait) > max_waits:
                    waits = list(si.on_wait)
                    extra, keep = waits[:-max_waits], waits[-max_waits:]
                    for ci in range(0, len(extra), max_waits):
                        nop = mybir.InstNoOp(
                            name=f"I-waitfix-{nc.next_id()}", ins=[], outs=[]
                        )
                        nop.engine = ins.engine
                        nop.sync_info = mybir.SyncInfo(
                            on_wait=extra[ci : ci + max_waits], on_update=[]
                        )
                        out.append(nop)
                        n_split += 1
                    si.on_wait = keep
                    changed = True
                out.append(ins)
            if changed:
                blk.instructions = out
    return n_split


def build_attention_nc():
    nc = bass.Bass(trn_type="TRN2")

    xt = nc.dram_tensor("xt", [BPC, D, S], BF16, kind="ExternalInput")
    wq = nc.dram_tensor("wq", [D, D], BF16, kind="ExternalInput")
    wk = nc.dram_tensor("wk", [D, D], BF16, kind="ExternalInput")
    wv = nc.dram_tensor("wv", [D, D], BF16, kind="ExternalInput")
    bq = nc.dram_tensor("bq", [D], F32, kind="ExternalInput")
    bk = nc.dram_tensor("bk", [D], F32, kind="ExternalInput")
    y = nc.dram_tensor("y", [BPC, D], F32, kind="ExternalOutput")

    with tile.TileContext(nc) as tc:
        with (
            tc.tile_pool(name="singles", bufs=1) as singles,
            tc.tile_pool(name="xT_pool", bufs=2) as xT_pool,
            tc.tile_pool(name="qkv_pool", bufs=2) as qkv_pool,
            tc.tile_pool(name="e_pool", bufs=3) as e_pool,
            tc.tile_pool(name="small_pool", bufs=4) as small_pool,
            tc.tile_pool(name="out_pool", bufs=2) as out_pool,
            tc.tile_pool(name="eps_pool", bufs=2, space="PSUM") as eps_pool,
        ):
            # ---- HAM warmup: dense dummy matmuls while the initial DMAs
            # are in flight, so the PE clock gate is already at 8/8 when
            # real work arrives. ----
            ones_bf = singles.tile([P, P], BF16, tag="ones_bf")
            nc.vector.memset(ones_bf[:], 1.0)
            zeros_bf = singles.tile([P, P], BF16, tag="zeros_bf")
            nc.vector.memset(zeros_bf[:], 0.0)
            with tc.tile_pool(name="warm_ps", bufs=1, space="PSUM") as wp:
                wm_ps = wp.tile([P, P], F32, name="wm_ps")
                for _ in range(14):
                    nc.tensor.matmul(
                        wm_ps[:], ones_bf[:], ones_bf[:], start=True, stop=True
                    )

            # prime the ScalarE exp table set off the critical path
            dummy = singles.tile([P, 1], F32, tag="dummy")
            nc.vector.memset(dummy[:], 0.0)
            dummy_o = singles.tile([P, 1], F32, tag="dummy_o")
            nc.scalar.activation(dummy_o[:], dummy[:], EXP)

            # ---- weights / constants (split across both HWDGE queues) ----
            wq_sb = singles.tile([P, CC, D], BF16, tag="wq")
            wk_sb = singles.tile([P, CC, D], BF16, tag="wk")
            wv_sb = singles.tile([P, CC, D], BF16, tag="wv")
            bq_sb = singles.tile([P, DT], F32, tag="bq")
            bk_sb = singles.tile([P, DT], F32, tag="bk")
            nc.sync.dma_start(wq_sb[:], wq.rearrange("(c p) d -> p c d", p=P))
            nc.sync.dma_start(wk_sb[:], wk.rearrange("(c p) d -> p c d", p=P))
            one_sb = singles.tile([1, 1], F32, tag="one")
            nc.vector.memset(one_sb[:], 1.0)

            # ---- prefetch both batches' x (host already transposed);
            # batch-0 chunks come right after the q/k weights so the first
            # projection matmuls are unblocked as early as possible ----
            xTs = []
            for b in range(BPC):
                xT = xT_pool.tile([P, CC, S], BF16, tag="xT", name=f"xT{b}")
                xt_r = xt[b].rearrange("(c p) s -> p c s", p=P)
                for sh in range(2):
                    for c in range(CC):
                        nc.sync.dma_start(
                            xT[:, c, ts(sh, S // 2)], xt_r[:, c, ts(sh, S // 2)]
                        )
                xTs.append(xT)
                if b == 0:
                    nc.sync.dma_start(
                        bq_sb[:], bq.rearrange("(t p) -> p t", p=P)
                    )
                    nc.sync.dma_start(
                        bk_sb[:], bk.rearrange("(t p) -> p t", p=P)
                    )
                    nc.sync.dma_start(
                        wv_sb[:], wv.rearrange("(c p) d -> p c d", p=P)
                    )

            def projections(b, pp, use_act):
                """bf16 projections, evacuated as fp8 q/k at x4 scale.
                qT stays planar [P, DT, S] (DoubleRow LDWEIGHTS needs
                non-degenerate pair strides); kT is interleaved [P, S, CC]
                so the DR moving operand's dependency bounding-box stays
                narrow.  Batch 0 evacuates q/k on the then-idle ScalarE;
                batch 1 (overlapping batch-0 attention) uses DVE only so
                ScalarE stays pure-exp."""
                xT = xTs[b]
                qT = qkv_pool.tile([P, DT, S], FP8, tag="qT", name=f"qT{b}")
                kT = qkv_pool.tile([P, S, CC], FP8, tag="kT", name=f"kT{b}")
                v = qkv_pool.tile([P, ST, D], BF16, tag="v", name=f"v{b}")
                for which in ("q", "k"):
                    w_sb = wq_sb if which == "q" else wk_sb
                    b_sb = bq_sb if which == "q" else bk_sb
                    for dt_ in range(DT):
                        for ns in range(NS):
                            ps = pp.tile([P, 512], F32, tag="qk", name="ps_qk")
                            for cc in range(CC):
                                nc.tensor.matmul(
                                    ps[:],
                                    w_sb[:, cc, ts(dt_, P)],
                                    xT[:, cc, ts(ns, 512)],
                                    start=(cc == 0),
                                    stop=(cc == CC - 1),
                                )
                            out_ap = (
                                qT[:, dt_, ts(ns, 512)] if which == "q"
                                else kT[:, ts(ns, 512), dt_ : dt_ + 1]
                            )
                            if use_act:
                                nc.scalar.activation(
                                    out_ap, ps[:], IDENT,
                                    bias=b_sb[:, dt_ : dt_ + 1], scale=QK_SCALE,
                                )
                            else:
                                nc.vector.tensor_scalar(
                                    out_ap, ps[:], QK_SCALE,
                                    b_sb[:, dt_ : dt_ + 1], MULT, ADD,
                                )
                for st in range(ST):
                    vps = pp.tile([P, 512], F32, tag="qk", name="ps_v")
                    for cc in range(CC):
                        nc.tensor.matmul(
                            vps[:, :D],
                            xT[:, cc, ts(st, P)],
                            wv_sb[:, cc, :],
                            start=(cc == 0),
                            stop=(cc == CC - 1),
                        )
                    nc.vector.tensor_copy(v[:, st, :], vps[:, :D])
                return qT, kT, v

            def attention(b, qT, kT, cp, pump=None, drain=True):
                """energy -> exp(+row-sum) -> w-weighted column-sum.

                Software-pipelined: tile t's colsum matmuls are emitted
                after tile t+2's energy matmuls so the PE never stalls
                waiting for w(t) = 1/Z(t).  The 4 colsum slices live in
                partition rows 0/32/64/96 of a single PSUM bank (via
                tile_position col-tiling); the accumulation group is opened
                by one zeroing matmul across all 128 partitions so the
                per-slice matmuls never clear each other's has_written
                bits."""
                colsum_sb = small_pool.tile([1, S], F32, tag="colsum_sb",
                                            name=f"colsum_sb{b}")
                cs_ps = cp.tile([P, 512], F32, name="cs_ps")
                # open the accumulation group: zero the whole bank
                nc.tensor.matmul(
                    cs_ps[:], zeros_bf[:], ones_bf[:, 0:1].to_broadcast((P, 512)),
                    start=True, stop=False, skip_group_check=True,
                )
                Es, wbs = [], []
                def emit_energy(t):
                    E = e_pool.tile([P, S], BF16, tag="E", name="E")
                    z2 = small_pool.tile([P, 2], F32, tag="z2", name="z2")
                    for h in range(2):
                        eps = eps_pool.tile([P, 1024], F32, tag="e", name="ps_e")
                        for n2 in range(2):
                            rhs = kT[
                                :, ds(h * 1024 + n2 * 512, 512), :
                            ].rearrange("p k c -> p c k")
                            nc.tensor.matmul(
                                eps[:, ts(n2, 512)],
                                qT[:, :, ts(t, P)],
                                rhs,
                                start=True, stop=True, perf_mode=DR,
                            )
                        nc.scalar.activation(
                            E[:, ts(h, 1024)],
                            eps[:],
                            EXP,
                            scale=EXP_SCALE,
                            accum_out=z2[:, h : h + 1],
                        )
                    zs = small_pool.tile([P, 1], F32, tag="zs", name="zs")
                    nc.vector.tensor_add(zs[:], z2[:, 0:1], z2[:, 1:2])
                    wf = small_pool.tile([P, 1], F32, tag="wf", name="wf")
                    nc.vector.reciprocal(wf[:], zs[:])
                    wb = small_pool.tile([P, 1], BF16, tag="wb", name="wb")
                    nc.vector.tensor_copy(wb[:], wf[:])
                    Es.append(E); wbs.append(wb)
                def emit_colsum(t):
                    last = t == ST - 1
                    for ns in range(NS):
                        nc.tensor.matmul(
                            cs_ps[32 * ns : 32 * ns + 1, :],
                            wbs[t][:],
                            Es[t][:, ts(ns, 512)],
                            start=False,
                            stop=last and ns == NS - 1,
                            tile_position=(0, 32 * ns),
                            skip_group_check=True,
                        )
                def pump1():
                    if pump:
                        pump.pop(0)()
                emit_energy(0)
                pump1()
                emit_energy(1)
                pump1()
                for t in range(2, ST):
                    emit_energy(t)
                    emit_colsum(t - 2)
                    pump1()
                emit_colsum(ST - 2)
                pump1()
                emit_colsum(ST - 1)
                pump1()
                if not drain:
                    return cs_ps
                for ns in range(NS):
                    nc.vector.tensor_copy(
                        colsum_sb[0:1, ts(ns, 512)],
                        cs_ps[32 * ns : 32 * ns + 1, :],
                    )
                return colsum_sb

            def final_matvec(b, colsum_sb, v, fp):
                # one PSUM bank: colT in cols 0..15, the out row after it
                fin_ps = fp.tile([P, 16 + D], F32, name="fin_ps")
                colT_ps = fin_ps[:, 0:ST]
                out_ps = fin_ps[0:1, ST : ST + D]
                for t in range(ST):
                    nc.tensor.matmul(
                        colT_ps[:, t : t + 1],
                        colsum_sb[0:1, ts(t, P)],
                        one_sb[0:1, 0:1],
                        start=(t == 0),
                        stop=(t == ST - 1),
                    )
                colT = small_pool.tile([P, ST], BF16, tag="colT")
                nc.vector.tensor_copy(colT[:], colT_ps[:])
                for t in range(ST):
                    nc.tensor.matmul(
                        out_ps[:],
                        colT[:, t : t + 1],
                        v[:, t, :],
                        start=(t == 0),
                        stop=(t == ST - 1),
                    )
                y_sb = out_pool.tile([1, D], F32, tag="y_sb")
                nc.vector.tensor_copy(y_sb[:], out_ps[:])
                nc.sync.dma_start(y[b : b + 1, :], y_sb[:])

            def fin_from_psum(b, cs_ps, v, fp):
                """b1 tail: per-quadrant drain (split DVE/ScalarE, both
                idle then) -> 4 fp32 transposes -> cast -> 4 matvec
                matmuls, so the four chains pipeline across engines
                instead of running as one serial pass."""
                colsum_sb = small_pool.tile([1, S], F32, tag="colsum_sb",
                                            name=f"colsum_sb{b}")
                fin_ps = fp.tile([P, 16 + D], F32, name="fin_ps")
                colT_ps = fin_ps[:, 0:ST]
                out_ps = fin_ps[0:1, ST : ST + D]
                colT = small_pool.tile([P, ST], BF16, tag="colT")
                for ns in range(NS):
                    if ns >= 2:
                        nc.scalar.copy(
                            colsum_sb[0:1, ts(ns, 512)],
                            cs_ps[32 * ns : 32 * ns + 1, :],
                        )
                    else:
                        nc.vector.tensor_copy(
                            colsum_sb[0:1, ts(ns, 512)],
                            cs_ps[32 * ns : 32 * ns + 1, :],
                        )
                    for t in range(4 * ns, 4 * ns + 4):
                        nc.tensor.matmul(
                            colT_ps[:, t : t + 1],
                            colsum_sb[0:1, ts(t, P)],
                            one_sb[0:1, 0:1],
                            start=(t == 0),
                            stop=(t == ST - 1),
                        )
                    nc.vector.tensor_copy(
                        colT[:, ds(4 * ns, 4)], colT_ps[:, ds(4 * ns, 4)]
                    )
                    for t in range(4 * ns, 4 * ns + 4):
                        nc.tensor.matmul(
                            out_ps[:],
                            colT[:, t : t + 1],
                            v[:, t, :],
                            start=(t == 0),
                            stop=(t == ST - 1),
                        )
                y_sb = out_pool.tile([1, D], F32, tag="y_sb")
                nc.vector.tensor_copy(y_sb[:], out_ps[:])
                nc.sync.dma_start(y[b : b + 1, :], y_sb[:])

            # Max-overlap phase order; PSUM bank budget (of 8):
            #   energy 4 (global pool) + colsum 1 + stream 2 + spare = 8.
            # batch-1 q/k projections are pumped INTO batch-0's attention
            # (1 psum per tile) so its first energy matmul - which waits
            # on ALL q evacuations via the planar qT bounding box - is
            # unblocked the moment batch-0's exp stream ends.
            with tc.tile_pool(name="proj_ps_0", bufs=2, space="PSUM") as pp0:
                q0, k0, v0 = projections(0, pp0, use_act=True)
            q1 = qkv_pool.tile([P, DT, S], FP8, tag="qT", name="qT1")
            k1 = qkv_pool.tile([P, S, CC], FP8, tag="kT", name="kT1")
            v1 = qkv_pool.tile([P, ST, D], BF16, tag="v", name="v1")
            with tc.tile_pool(name="stream", bufs=2, space="PSUM") as stp:
                units = []
                for which in ("q", "k"):
                    w_sb = wq_sb if which == "q" else wk_sb
                    b_sb = bq_sb if which == "q" else bk_sb
                    for dt_ in range(DT):
                        for ns in range(NS):
                            def unit(w_sb=w_sb, b_sb=b_sb, which=which,
                                     dt_=dt_, ns=ns):
                                ps = stp.tile([P, 512], F32, tag="st",
                                              name="st")
                                for cc in range(CC):
                                    nc.tensor.matmul(
                                        ps[:], w_sb[:, cc, ts(dt_, P)],
                                        xTs[1][:, cc, ts(ns, 512)],
                                        start=(cc == 0), stop=(cc == CC - 1),
                                    )
                                out_ap = (
                                    q1[:, dt_, ts(ns, 512)] if which == "q"
                                    else k1[:, ts(ns, 512), dt_ : dt_ + 1]
                                )
                                nc.vector.tensor_scalar(
                                    out_ap, ps[:], QK_SCALE,
                                    b_sb[:, dt_ : dt_ + 1], MULT, ADD,
                                )
                            units.append(unit)
                with tc.tile_pool(name="cs_ps_0", bufs=1, space="PSUM") as cp0:
                    cs0 = attention(0, q0, k0, cp0, pump=units)
                # v(b1) through the same stream pool; its DVE copies drain
                # during early batch-1 attention (v only needed at fin(b1))
                for st in range(ST):
                    vps = stp.tile([P, 512], F32, tag="st", name="st")
                    for cc in range(CC):
                        nc.tensor.matmul(
                            vps[:, :D], xTs[1][:, cc, ts(st, P)],
                            wv_sb[:, cc, :],
                            start=(cc == 0), stop=(cc == CC - 1),
                        )
                    nc.vector.tensor_copy(v1[:, st, :], vps[:, :D])
            with tc.tile_pool(name="fin_ps_0", bufs=1, space="PSUM") as fp0:
                final_matvec(0, cs0, v0, fp0)
                with tc.tile_pool(name="cs_ps_1", bufs=1, space="PSUM") as cp1:
                    cs_ps1 = attention(1, q1, k1, cp1, drain=False)
                    with tc.tile_pool(name="fin_ps_1", bufs=1,
                                      space="PSUM") as fp1:
                        fin_from_psum(1, cs_ps1, v1, fp1)

    _split_wide_waits(nc)
    return nc


_NC_CACHE = None


def _get_nc():
    global _NC_CACHE
    if _NC_CACHE is None:
        _NC_CACHE = build_attention_nc()
    return _NC_CACHE


def kernel(x, Wq, bq, Wk, bk, Wv, bv, _return_raw=False, _trace=False):
    x = np.asarray(x, dtype=np.float32)
    # pre-transpose on host: device wants the contraction dim on partitions
    xt_bf = np.ascontiguousarray(x.transpose(0, 2, 1)).astype(ml_dtypes.bfloat16)
    wq_bf = np.asarray(Wq, dtype=np.float32).astype(ml_dtypes.bfloat16)
    wk_bf = np.asarray(Wk, dtype=np.float32).astype(ml_dtypes.bfloat16)
    wv_bf = np.asarray(Wv, dtype=np.float32).astype(ml_dtypes.bfloat16)
    # device evac computes QK_SCALE*ps + bias_input -> send QK_SCALE*bias
    bq32 = np.ascontiguousarray(QK_SCALE * np.asarray(bq, dtype=np.float32))
    bk32 = np.ascontiguousarray(QK_SCALE * np.asarray(bk, dtype=np.float32))

    nc = _get_nc()
    in_maps = [
        {
            "xt": np.ascontiguousarray(xt_bf[i * BPC : (i + 1) * BPC]),
            "wq": wq_bf,
            "wk": wk_bf,
            "wv": wv_bf,
            "bq": bq32,
            "bk": bk32,
        }
        for i in range(N_CORES)
    ]
    res = run_bass_kernel_spmd(
        nc, in_maps, core_ids=list(range(N_CORES)), trace=_trace
    )
    out = np.concatenate([res.results[i]["y"] for i in range(N_CORES)], axis=0)
    out = out + S * np.asarray(bv, dtype=np.float32)[None, :]
    out = out.astype(np.float32)
    if _return_raw:
        return out, res
    return out
